# revision 34
# baseline (speedup 1.0000x reference)
"""BinaryXnorExceptOutliersLinear forward on 8 TRN2 NeuronCores.

out = x @ w_sim.T + bias, where w_sim binarizes non-outlier weights to
sign(w) * mean(|w| over non-outliers) and keeps outliers (|w - mean| >
1.6 * std, global scalar stats) at full precision.

Strategy (column-parallel on out_features, bf16 matmul):
  - host: transpose x -> xT [4096, 8192] in bf16 (replicated to all cores),
    shard weight along out_features (512/core, transposed to [4096, 512] f32).
  - device:
    A1: per-chunk Sum(w) on VectorE + Sum(w^2) on ScalarE while the w shard
        DMA streams in; cross-partition totals via gpsimd partition_all_reduce;
        tiny AllReduce #1 -> global mean/std -> [lower, upper] thresholds.
        During the collective window ScalarE precomputes sign(w) per chunk.
    A2: c = clamp(w, lower, upper) (VectorE), outlier-high count (VectorE,
        accum), nonout = (c == w) mask (GpSimd, accum), Sum|c| (ScalarE,
        accum).  Sum|w| over non-outliers = Sum|c| - upper*n_hi + lower*n_lo
        exactly, so a second tiny AllReduce of (n_hi, n_nonout, Sum|c|) gives
        the global binary_scale s.
    B:  w_sim = w + nonout * (s * sign(w) - w), built per chunk in bf16,
        interleaved with the start of the matmul phase.
    C:  dense bf16 matmul streaming xT k-slices (full TensorE rate), bias
        added during PSUM->SBUF eviction on ScalarE.
  - host: concatenate the per-core [512, 8192] outT shards, transpose back.
"""

import numpy as np
import ml_dtypes

import concourse.bass as bass
import concourse.mybir as mybir
from concourse.alu_op_type import AluOpType
from concourse.bass_utils import run_bass_kernel_spmd
from concourse.vector_clock import ScopedClock

import bass_rust
import concourse.tile as tile

F = mybir.ActivationFunctionType
FP32 = mybir.dt.float32
BF16 = mybir.dt.bfloat16
U8 = mybir.dt.uint8
X = mybir.AxisListType.X

N_CORES = 8
D_IN = 4096
D_OUT = 4096
TOK = 8192            # 4 * 2048 tokens
D_OUT_SH = D_OUT // N_CORES   # 512 out features per core
KC = D_IN // 128      # 32 k-chunks
MSUB = D_OUT_SH // 128  # 4 psum-partition chunks of out features
TOK_TILE = 512
N_TOKT = TOK // TOK_TILE  # 16
N_ELEM = D_OUT * D_IN     # full-weight element count for global stats
STD_K = 1.6


class _LegalTileContext(tile.TileContext):
    """TileContext that legalizes sem waits for this walrus build.

    The walrus here encodes a single wait slot per 64B instruction, so any
    instruction Tile annotates with N>1 sem waits fails codegen ("Too many
    sync wait commands").  Split the extras onto single-wait NOPs placed
    immediately before the instruction on the same engine, and do the same
    for the exit drain's global-clock waits.
    """

    def _add_instruction(self, inst):
        si = inst.sync_info
        if si is not None and si.on_wait and len(si.on_wait) > 1:
            waits = list(si.on_wait)
            for w in waits[:-1]:
                nop = bass_rust.InstNoOp(
                    text_hint="wait_split",
                    bass_nofuse=True,
                    name=self.nc.get_next_instruction_name(),
                    engine=inst.engine,
                    sync_info=mybir.SyncInfo(on_wait=[w], on_update=[]),
                )
                super()._add_instruction(nop)
            si.on_wait = waits[-1:]
            inst.sync_info = si
        super()._add_instruction(inst)

    def _drain_and_barrier(self, tick_clock, wait_clock):
        probe = self.nc.sync.nop(hint="drain_wait_probe", nofuse=True)
        wait_clock.add_sem_waits(
            probe.ins, ScopedClock({None: tick_clock.global_clock})
        )
        waits = list(probe.ins.sync_info.on_wait or []) if probe.ins.sync_info else []
        if len(waits) > 1:
            probe.ins.sync_info.on_wait = waits[:1]
            for w in waits[1:]:
                nop = self.nc.sync.nop(hint="drain_wait_split", nofuse=True)
                si = nop.ins.sync_info
                if si is None:
                    nop.ins.sync_info = mybir.SyncInfo(on_wait=[w], on_update=[])
                else:
                    si.on_wait = [w]
        self.nc.sync.drain()
        self.nc.all_engine_barrier()
        assert self.sems is not None
        popped = self.nc._tile_sem_poison_stack.pop()
        assert popped is self._sem_poison
        self.nc.clear_and_free_semaphores(list(self.sems.allocated().values()))
        self.nc.all_engine_barrier()


def _build_program():
    nc = bass.Bass()
    # x is staged host-side as [tt, k, p, j] so every [128, 512] tile is one
    # fully contiguous 128 KB DMA read.
    xt_in = nc.dram_tensor("xt", [N_TOKT * D_IN, TOK_TILE], BF16,
                           kind="ExternalInput")
    wt_in = nc.dram_tensor("wt", [D_IN, D_OUT_SH], FP32, kind="ExternalInput")
    b_in = nc.dram_tensor("bias", [128, MSUB], FP32, kind="ExternalInput")
    out_t = nc.dram_tensor("out", [D_OUT_SH, TOK], FP32, kind="ExternalOutput")

    groups = [list(range(N_CORES))]

    with _LegalTileContext(nc) as tc:
        with (
            tc.tile_pool(name="wraw", bufs=1) as wp,
            tc.tile_pool(name="wsim", bufs=1) as wsim_p,
            tc.tile_pool(name="sg", bufs=1) as sgp,
            tc.tile_pool(name="mbar", bufs=1) as mp,
            tc.tile_pool(name="consts", bufs=1) as cp,
            tc.tile_pool(name="stats", bufs=1) as st,
            tc.tile_pool(name="scrA", bufs=3) as spA,
            tc.tile_pool(name="scrB", bufs=3) as spB,
            tc.tile_pool(name="xs", bufs=12) as xp,
            tc.tile_pool(name="outs", bufs=4) as op,
            tc.tile_pool(name="dram", bufs=1, space="DRAM") as dram,
        ):
            # ---- inputs ---------------------------------------------------
            bias_sb = cp.tile([128, MSUB], FP32)
            nc.gpsimd.dma_start(bias_sb[:], b_in[:])
            ones_col = cp.tile([128, 1], FP32)
            nc.vector.memset(ones_col[:], 1.0)
            ones_row = cp.tile([1, 128], FP32)
            nc.vector.memset(ones_row[:], 1.0)
            ps_s_cm = tc.tile_pool(name="psum_s", bufs=1, space="PSUM")
            ps_s = ps_s_cm.__enter__()

            wt = []
            for k in range(KC):
                t = wp.tile([128, D_OUT_SH], FP32, tag=f"w{k}")
                nc.sync.dma_start(t[:], wt_in[k * 128:(k + 1) * 128, :])
                wt.append(t)

            # ---- phase A1: Sum(w) on V, Sum(w^2) on S, per chunk ----------
            accS = st.tile([128, KC], FP32)
            accSS = st.tile([128, KC], FP32)
            for k in range(KC):
                nc.vector.tensor_reduce(accS[:, k:k + 1], wt[k][:], axis=X,
                                        op=AluOpType.add)
                sq = spB.tile([128, D_OUT_SH], FP32, tag="sq")
                nc.scalar.activation(sq[:], wt[k][:], F.Square,
                                     accum_out=accSS[:, k:k + 1])

            acc2 = st.tile([128, 2], FP32)
            nc.vector.tensor_reduce(acc2[:, 0:1], accS[:], axis=X,
                                    op=AluOpType.add)
            nc.vector.tensor_reduce(acc2[:, 1:2], accSS[:], axis=X,
                                    op=AluOpType.add)
            p1 = ps_s.tile([1, 2], FP32, tag="p1")
            nc.tensor.matmul(p1[:], ones_col[:], acc2[:], start=True, stop=True)
            sb1 = st.tile([1, 2], FP32)
            nc.vector.tensor_copy(sb1[:], p1[:])

            bnc1 = dram.tile([1, 2], FP32)
            bnc1o = dram.tile([1, 2], FP32)
            nc.gpsimd.dma_start(bnc1[:], sb1[:])
            nc.gpsimd.collective_compute(
                "AllReduce", mybir.AluOpType.add,
                replica_groups=groups,
                ins=[bnc1.opt()], outs=[bnc1o.opt()],
            )
            gs1 = st.tile([1, 2], FP32)
            nc.gpsimd.dma_start(gs1[:], bnc1o[:])
            pb1 = ps_s.tile([128, 2], FP32, tag="pb1")
            nc.tensor.matmul(pb1[:], ones_row[:], gs1[:], start=True, stop=True)
            g1b = st.tile([128, 2], FP32)
            nc.vector.tensor_copy(g1b[:], pb1[:])

            # ---- sign(w) prepass on ScalarE (no stats deps; fills the
            # collective-wait window) ---------------------------------------
            sg = []
            for k in range(KC):
                t = sgp.tile([128, D_OUT_SH], BF16, tag=f"sg{k}")
                nc.scalar.activation(t[:], wt[k][:], F.Sign)
                sg.append(t)

            # ---- global scalar math (replicated on 128 partitions) --------
            # mean = S/N;  (N-1)*var = SS - S*mean, folded into the Sqrt's
            # scale so the serial chain is mean -> nv -> std -> {4 parallel}
            mean = st.tile([128, 1], FP32)
            nv = st.tile([128, 1], FP32)
            std = st.tile([128, 1], FP32)
            upper = st.tile([128, 1], FP32)
            lower = st.tile([128, 1], FP32)
            thr = st.tile([128, 1], FP32)
            nmean = st.tile([128, 1], FP32)
            nc.scalar.mul(mean[:], g1b[:, 0:1], 1.0 / N_ELEM)
            nc.vector.scalar_tensor_tensor(
                nv[:], g1b[:, 0:1], mean[:, 0:1], g1b[:, 1:2],
                AluOpType.mult, AluOpType.subtract)
            nc.scalar.activation(std[:], nv[:], F.Sqrt,
                                 scale=-1.0 / (N_ELEM - 1.0))
            nc.scalar.activation(upper[:], std[:], F.Identity,
                                 scale=STD_K, bias=mean[:, 0:1])
            nc.scalar.activation(lower[:], std[:], F.Identity,
                                 scale=-STD_K, bias=mean[:, 0:1])
            nc.scalar.mul(thr[:], std[:], STD_K)
            nc.scalar.mul(nmean[:], mean[:], -1.0)

            # ---- phase A2: c = clamp(w, lower, upper) (VectorE), outlier
            # mask m = (c != w) with accum -> n_out (VectorE stt-reduce),
            # Sum|c - mean| on ScalarE (accum).  Sum_nonout|w - mean| =
            # Sum|c - mean| - thr * n_out (exact: clamped outliers sit at
            # mean +- thr), equals Sum_nonout|w| up to ~1e-7.
            accNB = st.tile([128, KC], FP32)
            accAC = st.tile([128, KC], FP32)
            mout = []
            for k in range(KC):
                c = spA.tile([128, D_OUT_SH], FP32, tag="c")
                nc.vector.tensor_scalar(c[:], wt[k][:], lower[:, 0:1],
                                        upper[:, 0:1],
                                        op0=AluOpType.max, op1=AluOpType.min)
                m = mp.tile([128, D_OUT_SH], U8, tag=f"mb{k}")
                nc.vector.scalar_tensor_tensor(
                    m[:], c[:], 1.0, wt[k][:],
                    AluOpType.mult, AluOpType.not_equal,
                    accum_out=accNB[:, k:k + 1])
                mout.append(m)
                ac = spA.tile([128, D_OUT_SH], BF16, tag="ac")
                nc.scalar.activation(ac[:], c[:], F.Abs,
                                     bias=nmean[:, 0:1],
                                     accum_out=accAC[:, k:k + 1])

            acc3 = st.tile([128, 2], FP32)
            nc.vector.tensor_reduce(acc3[:, 0:1], accNB[:], axis=X,
                                    op=AluOpType.add)
            nc.vector.tensor_reduce(acc3[:, 1:2], accAC[:], axis=X,
                                    op=AluOpType.add)
            p2 = ps_s.tile([1, 2], FP32, tag="p2")
            nc.tensor.matmul(p2[:], ones_col[:], acc3[:], start=True, stop=True)
            sb2 = st.tile([1, 2], FP32)
            nc.vector.tensor_copy(sb2[:], p2[:])

            bnc2 = dram.tile([1, 2], FP32)
            bnc2o = dram.tile([1, 2], FP32)
            nc.gpsimd.dma_start(bnc2[:], sb2[:])
            nc.gpsimd.collective_compute(
                "AllReduce", mybir.AluOpType.add,
                replica_groups=groups,
                ins=[bnc2.opt()], outs=[bnc2o.opt()],
            )
            gs2 = st.tile([1, 2], FP32)
            nc.gpsimd.dma_start(gs2[:], bnc2o[:])
            pb2 = ps_s.tile([128, 2], FP32, tag="pb2")
            nc.tensor.matmul(pb2[:], ones_row[:], gs2[:], start=True, stop=True)
            g2b = st.tile([128, 2], FP32)
            nc.vector.tensor_copy(g2b[:], pb2[:])

            # binary_scale s = (Sum|c - mean| - thr * n_out) / n_nonout;
            # the matmul uses u = wsim / s (binaries exactly +-1) and s is
            # applied at PSUM eviction, so also build invs = 1 / s.
            n_out = g2b[:, 0:1]
            s_ac = g2b[:, 1:2]
            t1 = st.tile([128, 1], FP32)
            z1 = st.tile([128, 1], FP32)
            inv = st.tile([128, 1], FP32)
            rz1 = st.tile([128, 1], FP32)
            s_b = st.tile([128, 1], FP32)
            invs = st.tile([128, 1], FP32)
            nc.vector.scalar_tensor_tensor(
                z1[:], n_out, thr[:, 0:1], s_ac,
                AluOpType.mult, AluOpType.subtract)
            nc.vector.tensor_scalar(t1[:], n_out, -1.0, float(N_ELEM),
                                    op0=AluOpType.mult, op1=AluOpType.add)
            # invs first: it gates the B phase (s_b is only needed at
            # eviction time, well after the first matmuls).
            nc.vector.reciprocal(rz1[:], z1[:])
            nc.vector.scalar_tensor_tensor(
                invs[:], t1[:], -1.0, rz1[:],
                AluOpType.mult, AluOpType.mult)
            nc.vector.reciprocal(inv[:], t1[:])
            nc.vector.scalar_tensor_tensor(
                s_b[:], z1[:], -1.0, inv[:],
                AluOpType.mult, AluOpType.mult)
            ps_s_cm.__exit__(None, None, None)

            # ---- phase B: u = sign(w) + outlier * (w/s - sign(w)), bf16 ---
            # Non-outlier stationary weights are EXACTLY +-1 (zero mantissa),
            # which cuts PE-array multiplier power enough to avoid the P0
            # sustained-power downclock; s is re-applied at PSUM eviction.
            wsim = []
            for k in range(KC):
                t = wsim_p.tile([128, D_OUT_SH], BF16, name=f"wsim{k}",
                                tag=f"wsim{k}")
                wsim.append(t)
            for k in range(KC):
                d = spB.tile([128, D_OUT_SH], FP32, tag="d")
                dm = spB.tile([128, D_OUT_SH], FP32, tag="dm")
                # chunk 0 in two half-width slices so the first matmuls
                # (m=0/1) can launch ~1us earlier
                slices = ([slice(0, 256), slice(256, 512)] if k == 0
                          else [slice(0, D_OUT_SH)])
                for sl in slices:
                    nc.vector.scalar_tensor_tensor(
                        d[:, sl], wt[k][:, sl], invs[:, 0:1], sg[k][:, sl],
                        AluOpType.mult, AluOpType.subtract)
                    nc.vector.scalar_tensor_tensor(
                        dm[:, sl], d[:, sl], 1.0, mout[k][:, sl],
                        AluOpType.mult, AluOpType.mult)
                    nc.vector.tensor_tensor(wsim[k][:, sl], sg[k][:, sl],
                                            dm[:, sl], op=AluOpType.add)

            # ---- phase C: dense bf16 matmul -------------------------------
            def xt_dma(tt, k):
                t = xp.tile([128, TOK_TILE], BF16, tag="xt")
                r0 = (tt * KC + k) * 128
                nc.sync.dma_start(t[:], xt_in[r0:r0 + 128, :])
                return t

            def evict(pt, tt, m):
                ot = op.tile([128, TOK_TILE], FP32, name=f"ot_{tt}_{m}",
                             tag="ot")
                # Last two token tiles: spread evictions across ScalarE and
                # VectorE and the out-DMAs across the gpsimd and sync queues
                # so the kernel tail isn't serialized on one engine/queue.
                late = tt >= N_TOKT - 2
                if late and m % 2 == 1:
                    nc.vector.tensor_scalar(ot[:], pt[:], s_b[:, 0:1],
                                            bias_sb[:, m:m + 1],
                                            op0=AluOpType.mult,
                                            op1=AluOpType.add)
                else:
                    nc.scalar.activation(ot[:], pt[:], F.Identity,
                                         scale=s_b[:, 0:1],
                                         bias=bias_sb[:, m:m + 1])
                dma_eng = nc.sync if (late and m % 2 == 0) else nc.gpsimd
                dma_eng.dma_start(
                    out_t[m * 128:(m + 1) * 128,
                          tt * TOK_TILE:(tt + 1) * TOK_TILE], ot[:])

            with (
                tc.tile_pool(name="ops", bufs=2, space="PSUM") as pp,
            ):
                # The first two token tiles run k-interleaved on both PSUM
                # buffer sets (8 banks): consumption is then 8 matmuls per
                # wsim chunk (~2.1 us), matching VectorE's build rate so the
                # binarize never stalls TensorE.
                ps0 = [pp.tile([128, TOK_TILE], FP32, name=f"ps_0_{m}",
                               tag=f"ps{m}") for m in range(MSUB)]
                ps1 = [pp.tile([128, TOK_TILE], FP32, name=f"ps_1_{m}",
                               tag=f"ps{m}") for m in range(MSUB)]
                for k in range(KC):
                    xa = xt_dma(0, k)
                    xb = xt_dma(1, k)
                    for m in range(MSUB):
                        nc.tensor.matmul(ps0[m][:],
                                         wsim[k][:, m * 128:(m + 1) * 128],
                                         xa[:],
                                         start=(k == 0), stop=(k == KC - 1))
                    for m in range(MSUB):
                        nc.tensor.matmul(ps1[m][:],
                                         wsim[k][:, m * 128:(m + 1) * 128],
                                         xb[:],
                                         start=(k == 0), stop=(k == KC - 1))
                for m in range(MSUB):
                    evict(ps0[m], 0, m)
                for m in range(MSUB):
                    evict(ps1[m], 1, m)

                for tt in range(2, N_TOKT):
                    psum = [pp.tile([128, TOK_TILE], FP32, name=f"ps_{tt}_{m}",
                                    tag=f"ps{m}")
                            for m in range(MSUB)]
                    for k in range(KC):
                        xt_t = xt_dma(tt, k)
                        for m in range(MSUB):
                            nc.tensor.matmul(
                                psum[m][:],
                                wsim[k][:, m * 128:(m + 1) * 128],
                                xt_t[:],
                                start=(k == 0), stop=(k == KC - 1))
                    for m in range(MSUB):
                        evict(psum[m], tt, m)

    return nc


_NC_CACHE = None


def _get_program():
    global _NC_CACHE
    if _NC_CACHE is None:
        _NC_CACHE = _build_program()
    return _NC_CACHE


def _make_in_maps(x, weight, bias):
    # [tt, k, p, j] block layout: tile (tt, k) is contiguous in DRAM.
    xT = np.ascontiguousarray(
        x.reshape(N_TOKT, TOK_TILE, KC, 128).astype(ml_dtypes.bfloat16)
        .transpose(0, 2, 3, 1)).reshape(N_TOKT * D_IN, TOK_TILE)
    in_maps = []
    for c in range(N_CORES):
        o0 = c * D_OUT_SH
        wT_c = np.ascontiguousarray(weight[o0:o0 + D_OUT_SH, :].T)  # [D_IN, 512]
        b_c = np.ascontiguousarray(
            bias[o0:o0 + D_OUT_SH].reshape(MSUB, 128).T)  # [128, MSUB]
        in_maps.append({"xt": xT, "wt": wT_c, "bias": b_c})
    return in_maps


def kernel(x: np.ndarray, weight: np.ndarray, bias: np.ndarray) -> np.ndarray:
    nc = _get_program()
    in_maps = _make_in_maps(x, weight, bias)
    res = run_bass_kernel_spmd(nc, in_maps, list(range(N_CORES)))
    outT = np.concatenate([res.results[c]["out"] for c in range(N_CORES)], axis=0)
    return np.ascontiguousarray(outT.T).reshape(x.shape[0], x.shape[1], D_OUT)


# revision 38
# speedup vs baseline: 1.0203x; 1.0203x over previous
"""BinaryXnorExceptOutliersLinear forward on 8 TRN2 NeuronCores.

out = x @ w_sim.T + bias, where w_sim binarizes non-outlier weights to
sign(w) * mean(|w| over non-outliers) and keeps outliers (|w - mean| >
1.6 * std, global scalar stats) at full precision.

Strategy (column-parallel on out_features, bf16 matmul):
  - host: transpose x -> xT [4096, 8192] in bf16 (replicated to all cores),
    shard weight along out_features (512/core, transposed to [4096, 512] f32).
  - device:
    A1: per-chunk Sum(w) on VectorE + Sum(w^2) on ScalarE while the w shard
        DMA streams in; cross-partition totals via gpsimd partition_all_reduce;
        tiny AllReduce #1 -> global mean/std -> [lower, upper] thresholds.
        During the collective window ScalarE precomputes sign(w) per chunk.
    A2: c = clamp(w, lower, upper) (VectorE), outlier-high count (VectorE,
        accum), nonout = (c == w) mask (GpSimd, accum), Sum|c| (ScalarE,
        accum).  Sum|w| over non-outliers = Sum|c| - upper*n_hi + lower*n_lo
        exactly, so a second tiny AllReduce of (n_hi, n_nonout, Sum|c|) gives
        the global binary_scale s.
    B:  w_sim = w + nonout * (s * sign(w) - w), built per chunk in bf16,
        interleaved with the start of the matmul phase.
    C:  dense bf16 matmul streaming xT k-slices (full TensorE rate), bias
        added during PSUM->SBUF eviction on ScalarE.
  - host: concatenate the per-core [512, 8192] outT shards, transpose back.
"""

import numpy as np
import ml_dtypes

import concourse.bass as bass
import concourse.mybir as mybir
from concourse.alu_op_type import AluOpType
from concourse.bass_utils import run_bass_kernel_spmd
from concourse.vector_clock import ScopedClock

import bass_rust
import concourse.tile as tile

F = mybir.ActivationFunctionType
FP32 = mybir.dt.float32
BF16 = mybir.dt.bfloat16
U8 = mybir.dt.uint8
X = mybir.AxisListType.X

N_CORES = 8
D_IN = 4096
D_OUT = 4096
TOK = 8192            # 4 * 2048 tokens
D_OUT_SH = D_OUT // N_CORES   # 512 out features per core
KC = D_IN // 128      # 32 k-chunks
MSUB = D_OUT_SH // 128  # 4 psum-partition chunks of out features
TOK_TILE = 512
N_TOKT = TOK // TOK_TILE  # 16
N_ELEM = D_OUT * D_IN     # full-weight element count for global stats
STD_K = 1.6


class _LegalTileContext(tile.TileContext):
    """TileContext that legalizes sem waits for this walrus build.

    The walrus here encodes a single wait slot per 64B instruction, so any
    instruction Tile annotates with N>1 sem waits fails codegen ("Too many
    sync wait commands").  Split the extras onto single-wait NOPs placed
    immediately before the instruction on the same engine, and do the same
    for the exit drain's global-clock waits.
    """

    def _add_instruction(self, inst):
        si = inst.sync_info
        if si is not None and si.on_wait and len(si.on_wait) > 1:
            waits = list(si.on_wait)
            for w in waits[:-1]:
                nop = bass_rust.InstNoOp(
                    text_hint="wait_split",
                    bass_nofuse=True,
                    name=self.nc.get_next_instruction_name(),
                    engine=inst.engine,
                    sync_info=mybir.SyncInfo(on_wait=[w], on_update=[]),
                )
                super()._add_instruction(nop)
            si.on_wait = waits[-1:]
            inst.sync_info = si
        super()._add_instruction(inst)

    def _drain_and_barrier(self, tick_clock, wait_clock):
        probe = self.nc.sync.nop(hint="drain_wait_probe", nofuse=True)
        wait_clock.add_sem_waits(
            probe.ins, ScopedClock({None: tick_clock.global_clock})
        )
        waits = list(probe.ins.sync_info.on_wait or []) if probe.ins.sync_info else []
        if len(waits) > 1:
            probe.ins.sync_info.on_wait = waits[:1]
            for w in waits[1:]:
                nop = self.nc.sync.nop(hint="drain_wait_split", nofuse=True)
                si = nop.ins.sync_info
                if si is None:
                    nop.ins.sync_info = mybir.SyncInfo(on_wait=[w], on_update=[])
                else:
                    si.on_wait = [w]
        self.nc.sync.drain()
        self.nc.all_engine_barrier()
        assert self.sems is not None
        popped = self.nc._tile_sem_poison_stack.pop()
        assert popped is self._sem_poison
        self.nc.clear_and_free_semaphores(list(self.sems.allocated().values()))
        self.nc.all_engine_barrier()


def _build_program():
    nc = bass.Bass()
    # x is staged host-side as [tt, k, p, j] so every [128, 512] tile is one
    # fully contiguous 128 KB DMA read.
    xt_in = nc.dram_tensor("xt", [N_TOKT * D_IN, TOK_TILE], BF16,
                           kind="ExternalInput")
    wt_in = nc.dram_tensor("wt", [D_IN, D_OUT_SH], FP32, kind="ExternalInput")
    b_in = nc.dram_tensor("bias", [128, MSUB], FP32, kind="ExternalInput")
    out_t = nc.dram_tensor("out", [D_OUT_SH, TOK], FP32, kind="ExternalOutput")

    groups = [list(range(N_CORES))]

    with _LegalTileContext(nc) as tc:
        with (
            tc.tile_pool(name="wraw", bufs=1) as wp,
            tc.tile_pool(name="wsim", bufs=1) as wsim_p,
            tc.tile_pool(name="sg", bufs=1) as sgp,
            tc.tile_pool(name="mbar", bufs=1) as mp,
            tc.tile_pool(name="consts", bufs=1) as cp,
            tc.tile_pool(name="stats", bufs=1) as st,
            tc.tile_pool(name="scrA", bufs=3) as spA,
            tc.tile_pool(name="scrB", bufs=3) as spB,
            tc.tile_pool(name="xs", bufs=12) as xp,
            tc.tile_pool(name="outs", bufs=4) as op,
            tc.tile_pool(name="dram", bufs=1, space="DRAM") as dram,
        ):
            # ---- inputs ---------------------------------------------------
            bias_sb = cp.tile([128, MSUB], FP32)
            nc.gpsimd.dma_start(bias_sb[:], b_in[:])
            ones_col = cp.tile([128, 1], FP32)
            nc.vector.memset(ones_col[:], 1.0)
            ones_row = cp.tile([1, 128], FP32)
            nc.vector.memset(ones_row[:], 1.0)
            ones8 = cp.tile([N_CORES, 128], FP32)
            nc.vector.memset(ones8[:], 1.0)
            ps_s_cm = tc.tile_pool(name="psum_s", bufs=1, space="PSUM")
            ps_s = ps_s_cm.__enter__()

            wt = []
            for k in range(KC):
                t = wp.tile([128, D_OUT_SH], FP32, tag=f"w{k}")
                nc.sync.dma_start(t[:], wt_in[k * 128:(k + 1) * 128, :])
                wt.append(t)

            # ---- phase A1: Sum(w) on V, Sum(w^2) on S, per chunk ----------
            accS = st.tile([128, KC], FP32)
            accSS = st.tile([128, KC], FP32)
            for k in range(KC):
                nc.vector.tensor_reduce(accS[:, k:k + 1], wt[k][:], axis=X,
                                        op=AluOpType.add)
                sq = spB.tile([128, D_OUT_SH], FP32, tag="sq")
                nc.scalar.activation(sq[:], wt[k][:], F.Square,
                                     accum_out=accSS[:, k:k + 1])

            acc2 = st.tile([128, 2], FP32)
            nc.vector.tensor_reduce(acc2[:, 0:1], accS[:], axis=X,
                                    op=AluOpType.add)
            nc.vector.tensor_reduce(acc2[:, 1:2], accSS[:], axis=X,
                                    op=AluOpType.add)
            p1 = ps_s.tile([1, 2], FP32, tag="p1")
            nc.tensor.matmul(p1[:], ones_col[:], acc2[:], start=True, stop=True)
            sb1 = st.tile([1, 2], FP32)
            nc.vector.tensor_copy(sb1[:], p1[:])

            bnc1 = dram.tile([1, 2], FP32)
            bnc1o = dram.tile([N_CORES, 2], FP32)
            nc.gpsimd.dma_start(bnc1[:], sb1[:])
            nc.gpsimd.collective_compute(
                "AllGather", mybir.AluOpType.bypass,
                replica_groups=groups,
                ins=[bnc1.opt()], outs=[bnc1o.opt()],
            )
            gs1 = st.tile([N_CORES, 2], FP32)
            nc.gpsimd.dma_start(gs1[:], bnc1o[:])
            # reduce over the 8 gathered rows AND broadcast to 128 partitions
            # in one matmul: ones8.T @ gs1
            pb1 = ps_s.tile([128, 2], FP32, tag="pb1")
            nc.tensor.matmul(pb1[:], ones8[:], gs1[:], start=True, stop=True)
            g1b = st.tile([128, 2], FP32)
            nc.vector.tensor_copy(g1b[:], pb1[:])

            # ---- sign(w) prepass on ScalarE (no stats deps; fills the
            # collective-wait window) ---------------------------------------
            sg = []
            for k in range(KC):
                t = sgp.tile([128, D_OUT_SH], BF16, tag=f"sg{k}")
                nc.scalar.activation(t[:], wt[k][:], F.Sign)
                sg.append(t)

            # ---- global scalar math (replicated on 128 partitions) --------
            # mean = S/N;  (N-1)*var = SS - S*mean, folded into the Sqrt's
            # scale so the serial chain is mean -> nv -> std -> {4 parallel}
            mean = st.tile([128, 1], FP32)
            nv = st.tile([128, 1], FP32)
            std = st.tile([128, 1], FP32)
            upper = st.tile([128, 1], FP32)
            lower = st.tile([128, 1], FP32)
            thr = st.tile([128, 1], FP32)
            nmean = st.tile([128, 1], FP32)
            nc.scalar.mul(mean[:], g1b[:, 0:1], 1.0 / N_ELEM)
            nc.vector.scalar_tensor_tensor(
                nv[:], g1b[:, 0:1], mean[:, 0:1], g1b[:, 1:2],
                AluOpType.mult, AluOpType.subtract)
            nc.scalar.activation(std[:], nv[:], F.Sqrt,
                                 scale=-1.0 / (N_ELEM - 1.0))
            nc.scalar.activation(upper[:], std[:], F.Identity,
                                 scale=STD_K, bias=mean[:, 0:1])
            nc.scalar.activation(lower[:], std[:], F.Identity,
                                 scale=-STD_K, bias=mean[:, 0:1])
            nc.scalar.mul(thr[:], std[:], STD_K)
            nc.scalar.mul(nmean[:], mean[:], -1.0)

            # ---- phase A2: c = clamp(w, lower, upper) (VectorE), outlier
            # mask m = (c != w) with accum -> n_out (VectorE stt-reduce),
            # Sum|c - mean| on ScalarE (accum).  Sum_nonout|w - mean| =
            # Sum|c - mean| - thr * n_out (exact: clamped outliers sit at
            # mean +- thr), equals Sum_nonout|w| up to ~1e-7.
            accNB = st.tile([128, KC], FP32)
            accAC = st.tile([128, KC], FP32)
            mout = []
            for k in range(KC):
                c = spA.tile([128, D_OUT_SH], FP32, tag="c")
                nc.vector.tensor_scalar(c[:], wt[k][:], lower[:, 0:1],
                                        upper[:, 0:1],
                                        op0=AluOpType.max, op1=AluOpType.min)
                m = mp.tile([128, D_OUT_SH], U8, tag=f"mb{k}")
                nc.vector.scalar_tensor_tensor(
                    m[:], c[:], 1.0, wt[k][:],
                    AluOpType.mult, AluOpType.not_equal,
                    accum_out=accNB[:, k:k + 1])
                mout.append(m)
                ac = spA.tile([128, D_OUT_SH], BF16, tag="ac")
                nc.scalar.activation(ac[:], c[:], F.Abs,
                                     bias=nmean[:, 0:1],
                                     accum_out=accAC[:, k:k + 1])

            acc3 = st.tile([128, 2], FP32)
            nc.vector.tensor_reduce(acc3[:, 0:1], accNB[:], axis=X,
                                    op=AluOpType.add)
            nc.vector.tensor_reduce(acc3[:, 1:2], accAC[:], axis=X,
                                    op=AluOpType.add)
            p2 = ps_s.tile([1, 2], FP32, tag="p2")
            nc.tensor.matmul(p2[:], ones_col[:], acc3[:], start=True, stop=True)
            sb2 = st.tile([1, 2], FP32)
            nc.vector.tensor_copy(sb2[:], p2[:])

            bnc2 = dram.tile([1, 2], FP32)
            bnc2o = dram.tile([N_CORES, 2], FP32)
            nc.gpsimd.dma_start(bnc2[:], sb2[:])
            nc.gpsimd.collective_compute(
                "AllGather", mybir.AluOpType.bypass,
                replica_groups=groups,
                ins=[bnc2.opt()], outs=[bnc2o.opt()],
            )
            gs2 = st.tile([N_CORES, 2], FP32)
            nc.gpsimd.dma_start(gs2[:], bnc2o[:])
            pb2 = ps_s.tile([128, 2], FP32, tag="pb2")
            nc.tensor.matmul(pb2[:], ones8[:], gs2[:], start=True, stop=True)
            g2b = st.tile([128, 2], FP32)
            nc.vector.tensor_copy(g2b[:], pb2[:])

            # binary_scale s = (Sum|c - mean| - thr * n_out) / n_nonout;
            # the matmul uses u = wsim / s (binaries exactly +-1) and s is
            # applied at PSUM eviction, so also build invs = 1 / s.
            n_out = g2b[:, 0:1]
            s_ac = g2b[:, 1:2]
            t1 = st.tile([128, 1], FP32)
            z1 = st.tile([128, 1], FP32)
            inv = st.tile([128, 1], FP32)
            rz1 = st.tile([128, 1], FP32)
            s_b = st.tile([128, 1], FP32)
            invs = st.tile([128, 1], FP32)
            nc.vector.scalar_tensor_tensor(
                z1[:], n_out, thr[:, 0:1], s_ac,
                AluOpType.mult, AluOpType.subtract)
            nc.vector.tensor_scalar(t1[:], n_out, -1.0, float(N_ELEM),
                                    op0=AluOpType.mult, op1=AluOpType.add)
            # invs first: it gates the B phase (s_b is only needed at
            # eviction time, well after the first matmuls).
            nc.vector.reciprocal(rz1[:], z1[:])
            nc.vector.scalar_tensor_tensor(
                invs[:], t1[:], -1.0, rz1[:],
                AluOpType.mult, AluOpType.mult)
            nc.vector.reciprocal(inv[:], t1[:])
            nc.vector.scalar_tensor_tensor(
                s_b[:], z1[:], -1.0, inv[:],
                AluOpType.mult, AluOpType.mult)
            ps_s_cm.__exit__(None, None, None)

            # ---- phase B: u = sign(w) + outlier * (w/s - sign(w)), bf16 ---
            # Non-outlier stationary weights are EXACTLY +-1 (zero mantissa),
            # which cuts PE-array multiplier power enough to avoid the P0
            # sustained-power downclock; s is re-applied at PSUM eviction.
            wsim = []
            for k in range(KC):
                t = wsim_p.tile([128, D_OUT_SH], BF16, name=f"wsim{k}",
                                tag=f"wsim{k}")
                wsim.append(t)
            for k in range(KC):
                d = spB.tile([128, D_OUT_SH], FP32, tag="d")
                dm = spB.tile([128, D_OUT_SH], FP32, tag="dm")
                # chunk 0 in two half-width slices so the first matmuls
                # (m=0/1) can launch ~1us earlier
                slices = ([slice(0, 256), slice(256, 512)] if k == 0
                          else [slice(0, D_OUT_SH)])
                for sl in slices:
                    nc.vector.scalar_tensor_tensor(
                        d[:, sl], wt[k][:, sl], invs[:, 0:1], sg[k][:, sl],
                        AluOpType.mult, AluOpType.subtract)
                    nc.vector.scalar_tensor_tensor(
                        dm[:, sl], d[:, sl], 1.0, mout[k][:, sl],
                        AluOpType.mult, AluOpType.mult)
                    nc.vector.tensor_tensor(wsim[k][:, sl], sg[k][:, sl],
                                            dm[:, sl], op=AluOpType.add)

            # ---- phase C: dense bf16 matmul -------------------------------
            def xt_dma(tt, k):
                t = xp.tile([128, TOK_TILE], BF16, tag="xt")
                r0 = (tt * KC + k) * 128
                nc.sync.dma_start(t[:], xt_in[r0:r0 + 128, :])
                return t

            def evict(pt, tt, m):
                ot = op.tile([128, TOK_TILE], FP32, name=f"ot_{tt}_{m}",
                             tag="ot")
                # Last two token tiles: spread evictions across ScalarE and
                # VectorE and the out-DMAs across the gpsimd and sync queues
                # so the kernel tail isn't serialized on one engine/queue.
                late = tt >= N_TOKT - 2
                if late and m % 2 == 1:
                    nc.vector.tensor_scalar(ot[:], pt[:], s_b[:, 0:1],
                                            bias_sb[:, m:m + 1],
                                            op0=AluOpType.mult,
                                            op1=AluOpType.add)
                else:
                    nc.scalar.activation(ot[:], pt[:], F.Identity,
                                         scale=s_b[:, 0:1],
                                         bias=bias_sb[:, m:m + 1])
                # Keep the gpsimd queue empty at the end so its ~7us DRAIN
                # (TileContext exit) hides under the main phase.
                if late:
                    dma_eng = nc.sync if m < 2 else nc.scalar
                else:
                    dma_eng = nc.gpsimd
                dma_eng.dma_start(
                    out_t[m * 128:(m + 1) * 128,
                          tt * TOK_TILE:(tt + 1) * TOK_TILE], ot[:])

            with (
                tc.tile_pool(name="ops", bufs=2, space="PSUM") as pp,
            ):
                # The first two token tiles run k-interleaved on both PSUM
                # buffer sets (8 banks): consumption is then 8 matmuls per
                # wsim chunk (~2.1 us), matching VectorE's build rate so the
                # binarize never stalls TensorE.
                ps0 = [pp.tile([128, TOK_TILE], FP32, name=f"ps_0_{m}",
                               tag=f"ps{m}") for m in range(MSUB)]
                ps1 = [pp.tile([128, TOK_TILE], FP32, name=f"ps_1_{m}",
                               tag=f"ps{m}") for m in range(MSUB)]
                for k in range(KC):
                    xa = xt_dma(0, k)
                    xb = xt_dma(1, k)
                    for m in range(MSUB):
                        nc.tensor.matmul(ps0[m][:],
                                         wsim[k][:, m * 128:(m + 1) * 128],
                                         xa[:],
                                         start=(k == 0), stop=(k == KC - 1))
                    for m in range(MSUB):
                        nc.tensor.matmul(ps1[m][:],
                                         wsim[k][:, m * 128:(m + 1) * 128],
                                         xb[:],
                                         start=(k == 0), stop=(k == KC - 1))
                for m in range(MSUB):
                    evict(ps0[m], 0, m)
                for m in range(MSUB):
                    evict(ps1[m], 1, m)

                for tt in range(2, N_TOKT):
                    psum = [pp.tile([128, TOK_TILE], FP32, name=f"ps_{tt}_{m}",
                                    tag=f"ps{m}")
                            for m in range(MSUB)]
                    for k in range(KC):
                        xt_t = xt_dma(tt, k)
                        for m in range(MSUB):
                            nc.tensor.matmul(
                                psum[m][:],
                                wsim[k][:, m * 128:(m + 1) * 128],
                                xt_t[:],
                                start=(k == 0), stop=(k == KC - 1))
                    for m in range(MSUB):
                        evict(psum[m], tt, m)

    return nc


_NC_CACHE = None


def _get_program():
    global _NC_CACHE
    if _NC_CACHE is None:
        _NC_CACHE = _build_program()
    return _NC_CACHE


def _make_in_maps(x, weight, bias):
    # [tt, k, p, j] block layout: tile (tt, k) is contiguous in DRAM.
    xT = np.ascontiguousarray(
        x.reshape(N_TOKT, TOK_TILE, KC, 128).astype(ml_dtypes.bfloat16)
        .transpose(0, 2, 3, 1)).reshape(N_TOKT * D_IN, TOK_TILE)
    in_maps = []
    for c in range(N_CORES):
        o0 = c * D_OUT_SH
        wT_c = np.ascontiguousarray(weight[o0:o0 + D_OUT_SH, :].T)  # [D_IN, 512]
        b_c = np.ascontiguousarray(
            bias[o0:o0 + D_OUT_SH].reshape(MSUB, 128).T)  # [128, MSUB]
        in_maps.append({"xt": xT, "wt": wT_c, "bias": b_c})
    return in_maps


def kernel(x: np.ndarray, weight: np.ndarray, bias: np.ndarray) -> np.ndarray:
    nc = _get_program()
    in_maps = _make_in_maps(x, weight, bias)
    res = run_bass_kernel_spmd(nc, in_maps, list(range(N_CORES)))
    outT = np.concatenate([res.results[c]["out"] for c in range(N_CORES)], axis=0)
    return np.ascontiguousarray(outT.T).reshape(x.shape[0], x.shape[1], D_OUT)


# revision 42
# speedup vs baseline: 1.0526x; 1.0317x over previous
"""BinaryXnorExceptOutliersLinear forward on 8 TRN2 NeuronCores.

out = x @ w_sim.T + bias, where w_sim binarizes non-outlier weights to
sign(w) * mean(|w| over non-outliers) and keeps outliers (|w - mean| >
1.6 * std, global scalar stats) at full precision.

Strategy (column-parallel on out_features, bf16 matmul):
  - host: transpose x -> xT [4096, 8192] in bf16 (replicated to all cores),
    shard weight along out_features (512/core, transposed to [4096, 512] f32).
  - device:
    A1: per-chunk Sum(w) on VectorE + Sum(w^2) on ScalarE while the w shard
        DMA streams in; cross-partition totals via gpsimd partition_all_reduce;
        tiny AllReduce #1 -> global mean/std -> [lower, upper] thresholds.
        During the collective window ScalarE precomputes sign(w) per chunk.
    A2: c = clamp(w, lower, upper) (VectorE), outlier-high count (VectorE,
        accum), nonout = (c == w) mask (GpSimd, accum), Sum|c| (ScalarE,
        accum).  Sum|w| over non-outliers = Sum|c| - upper*n_hi + lower*n_lo
        exactly, so a second tiny AllReduce of (n_hi, n_nonout, Sum|c|) gives
        the global binary_scale s.
    B:  w_sim = w + nonout * (s * sign(w) - w), built per chunk in bf16,
        interleaved with the start of the matmul phase.
    C:  dense bf16 matmul streaming xT k-slices (full TensorE rate), bias
        added during PSUM->SBUF eviction on ScalarE.
  - host: concatenate the per-core [512, 8192] outT shards, transpose back.
"""

import numpy as np
import ml_dtypes

import concourse.bass as bass
import concourse.mybir as mybir
from concourse.alu_op_type import AluOpType
from concourse.bass_utils import run_bass_kernel_spmd
from concourse.vector_clock import ScopedClock

import bass_rust
import concourse.tile as tile

F = mybir.ActivationFunctionType
FP32 = mybir.dt.float32
BF16 = mybir.dt.bfloat16
U8 = mybir.dt.uint8
X = mybir.AxisListType.X

N_CORES = 8
D_IN = 4096
D_OUT = 4096
TOK = 8192            # 4 * 2048 tokens
D_OUT_SH = D_OUT // N_CORES   # 512 out features per core
KC = D_IN // 128      # 32 k-chunks
MSUB = D_OUT_SH // 128  # 4 psum-partition chunks of out features
TOK_TILE = 512
N_TOKT = TOK // TOK_TILE  # 16
N_ELEM = D_OUT * D_IN     # full-weight element count for global stats
STD_K = 1.6


class _LegalTileContext(tile.TileContext):
    """TileContext that legalizes sem waits for this walrus build.

    The walrus here encodes a single wait slot per 64B instruction, so any
    instruction Tile annotates with N>1 sem waits fails codegen ("Too many
    sync wait commands").  Split the extras onto single-wait NOPs placed
    immediately before the instruction on the same engine, and do the same
    for the exit drain's global-clock waits.
    """

    def _add_instruction(self, inst):
        si = inst.sync_info
        if si is not None and si.on_wait and len(si.on_wait) > 1:
            waits = list(si.on_wait)
            for w in waits[:-1]:
                nop = bass_rust.InstNoOp(
                    text_hint="wait_split",
                    bass_nofuse=True,
                    name=self.nc.get_next_instruction_name(),
                    engine=inst.engine,
                    sync_info=mybir.SyncInfo(on_wait=[w], on_update=[]),
                )
                super()._add_instruction(nop)
            si.on_wait = waits[-1:]
            inst.sync_info = si
        super()._add_instruction(inst)

    def _drain_and_barrier(self, tick_clock, wait_clock):
        probe = self.nc.sync.nop(hint="drain_wait_probe", nofuse=True)
        wait_clock.add_sem_waits(
            probe.ins, ScopedClock({None: tick_clock.global_clock})
        )
        waits = list(probe.ins.sync_info.on_wait or []) if probe.ins.sync_info else []
        if len(waits) > 1:
            probe.ins.sync_info.on_wait = waits[:1]
            for w in waits[1:]:
                nop = self.nc.sync.nop(hint="drain_wait_split", nofuse=True)
                si = nop.ins.sync_info
                if si is None:
                    nop.ins.sync_info = mybir.SyncInfo(on_wait=[w], on_update=[])
                else:
                    si.on_wait = [w]
        self.nc.sync.drain()
        self.nc.all_engine_barrier()
        assert self.sems is not None
        popped = self.nc._tile_sem_poison_stack.pop()
        assert popped is self._sem_poison
        self.nc.clear_and_free_semaphores(list(self.sems.allocated().values()))
        self.nc.all_engine_barrier()


def _build_program():
    nc = bass.Bass()
    # x is staged host-side as [tt, k, p, j] so every [128, 512] tile is one
    # fully contiguous 128 KB DMA read.
    xt_in = nc.dram_tensor("xt", [N_TOKT * D_IN, TOK_TILE], BF16,
                           kind="ExternalInput")
    wt_in = nc.dram_tensor("wt", [D_IN, D_OUT_SH], FP32, kind="ExternalInput")
    b_in = nc.dram_tensor("bias", [128, MSUB], FP32, kind="ExternalInput")
    out_t = nc.dram_tensor("out", [D_OUT_SH, TOK], FP32, kind="ExternalOutput")

    groups = [list(range(N_CORES))]

    with _LegalTileContext(nc) as tc:
        with (
            tc.tile_pool(name="wraw", bufs=1) as wp,
            tc.tile_pool(name="wsim", bufs=1) as wsim_p,
            tc.tile_pool(name="sg", bufs=1) as sgp,
            tc.tile_pool(name="mbar", bufs=1) as mp,
            tc.tile_pool(name="consts", bufs=1) as cp,
            tc.tile_pool(name="stats", bufs=1) as st,
            tc.tile_pool(name="scrA", bufs=3) as spA,
            tc.tile_pool(name="scrB", bufs=2) as spB,
            tc.tile_pool(name="xs", bufs=10) as xp,
            tc.tile_pool(name="outs", bufs=4) as op,
            tc.tile_pool(name="dram", bufs=1, space="DRAM") as dram,
        ):
            # ---- inputs ---------------------------------------------------
            bias_sb = cp.tile([128, MSUB], FP32)
            nc.gpsimd.dma_start(bias_sb[:], b_in[:])
            ones_col = cp.tile([128, 1], FP32)
            nc.vector.memset(ones_col[:], 1.0)
            ones_row = cp.tile([1, 128], FP32)
            nc.vector.memset(ones_row[:], 1.0)
            ones8 = cp.tile([N_CORES, 128], FP32)
            nc.vector.memset(ones8[:], 1.0)
            ones_bf = cp.tile([128, 1], BF16)
            nc.vector.memset(ones_bf[:], 1.0)
            ps_s_cm = tc.tile_pool(name="psum_s", bufs=1, space="PSUM")
            ps_s = ps_s_cm.__enter__()

            wt = []
            for k in range(KC):
                t = wp.tile([128, D_OUT_SH], FP32, tag=f"w{k}")
                nc.sync.dma_start(t[:], wt_in[k * 128:(k + 1) * 128, :])
                wt.append(t)

            # ---- phase A1: Sum(w) on V, Sum(w^2) on S, per chunk ----------
            accS = st.tile([128, KC], FP32)
            accSS = st.tile([128, KC], FP32)
            for k in range(KC):
                nc.vector.tensor_reduce(accS[:, k:k + 1], wt[k][:], axis=X,
                                        op=AluOpType.add)
                sq = spB.tile([128, D_OUT_SH], FP32, tag="sq")
                nc.scalar.activation(sq[:], wt[k][:], F.Square,
                                     accum_out=accSS[:, k:k + 1])

            acc2 = st.tile([128, 2], FP32)
            nc.vector.tensor_reduce(acc2[:, 0:1], accS[:], axis=X,
                                    op=AluOpType.add)
            nc.vector.tensor_reduce(acc2[:, 1:2], accSS[:], axis=X,
                                    op=AluOpType.add)
            p1 = ps_s.tile([1, 2], FP32, tag="p1")
            nc.tensor.matmul(p1[:], ones_col[:], acc2[:], start=True, stop=True)
            sb1 = st.tile([1, 2], FP32)
            nc.vector.tensor_copy(sb1[:], p1[:])

            bnc1 = dram.tile([1, 2], FP32)
            bnc1o = dram.tile([N_CORES, 2], FP32)
            nc.gpsimd.dma_start(bnc1[:], sb1[:])
            nc.gpsimd.collective_compute(
                "AllGather", mybir.AluOpType.bypass,
                replica_groups=groups,
                ins=[bnc1.opt()], outs=[bnc1o.opt()],
            )
            gs1 = st.tile([N_CORES, 2], FP32)
            nc.gpsimd.dma_start(gs1[:], bnc1o[:])
            # reduce over the 8 gathered rows AND broadcast to 128 partitions
            # in one matmul: ones8.T @ gs1
            pb1 = ps_s.tile([128, 2], FP32, tag="pb1")
            nc.tensor.matmul(pb1[:], ones8[:], gs1[:], start=True, stop=True)
            g1b = st.tile([128, 2], FP32)
            nc.vector.tensor_copy(g1b[:], pb1[:])

            # ---- sign(w) prepass on ScalarE (no stats deps; fills the
            # collective-wait window) ---------------------------------------
            sg = []
            for k in range(KC):
                t = sgp.tile([128, D_OUT_SH], BF16, tag=f"sg{k}")
                nc.scalar.activation(t[:], wt[k][:], F.Sign)
                sg.append(t)

            # ---- global scalar math (replicated on 128 partitions) --------
            # nmean first: it alone gates the A2 aw-pass on ScalarE; the
            # thr chain (mean -> nv -> std -> thr) overlaps the first chunks.
            mean = st.tile([128, 1], FP32)
            nv = st.tile([128, 1], FP32)
            std = st.tile([128, 1], FP32)
            thr = st.tile([128, 1], FP32)
            nmean = st.tile([128, 1], FP32)
            nc.scalar.mul(nmean[:], g1b[:, 0:1], -1.0 / N_ELEM)
            nc.scalar.mul(mean[:], g1b[:, 0:1], 1.0 / N_ELEM)
            nc.vector.scalar_tensor_tensor(
                nv[:], g1b[:, 0:1], mean[:, 0:1], g1b[:, 1:2],
                AluOpType.mult, AluOpType.subtract)
            nc.scalar.activation(std[:], nv[:], F.Sqrt,
                                 scale=-1.0 / (N_ELEM - 1.0))
            nc.scalar.mul(thr[:], std[:], STD_K)

            # ---- phase A2: aw = |w - mean| (ScalarE, no accum), then two
            # PLAIN VectorE tensor_scalar passes: ca = min(aw, thr) and the
            # outlier mask m = (aw > thr), both bf16.  The two global sums
            # (Sum ca and n_out) accumulate on the otherwise-idle TensorE as
            # ones.T @ {ca, m} column sums into [1, 512] PSUM rows.
            # Sum_nonout|w - mean| = Sum(ca) - thr * n_out (exact: clamped
            # outliers contribute thr each), equals Sum_nonout|w| to ~1e-7.
            cs_ca = ps_s.tile([1, TOK_TILE], FP32, tag="csca")
            cs_m = ps_s.tile([1, TOK_TILE], FP32, tag="csm")
            mout = []
            for k in range(KC):
                aw = spA.tile([128, D_OUT_SH], FP32, tag="aw")
                nc.scalar.activation(aw[:], wt[k][:], F.Abs,
                                     bias=nmean[:, 0:1])
                ca = spA.tile([128, D_OUT_SH], BF16, tag="ca")
                nc.vector.tensor_scalar(ca[:], aw[:], thr[:, 0:1], 0.0,
                                        op0=AluOpType.min, op1=AluOpType.add)
                m = mp.tile([128, D_OUT_SH], BF16, tag=f"mb{k}")
                nc.vector.tensor_scalar(m[:], aw[:], thr[:, 0:1], 0.0,
                                        op0=AluOpType.is_gt, op1=AluOpType.add)
                mout.append(m)
                nc.tensor.matmul(cs_ca[:], ones_bf[:], ca[:],
                                 start=(k == 0), stop=(k == KC - 1))
                nc.tensor.matmul(cs_m[:], ones_bf[:], m[:],
                                 start=(k == 0), stop=(k == KC - 1))

            sb2 = st.tile([1, 2], FP32)
            nc.vector.tensor_reduce(sb2[:, 0:1], cs_m[:], axis=X,
                                    op=AluOpType.add)
            nc.vector.tensor_reduce(sb2[:, 1:2], cs_ca[:], axis=X,
                                    op=AluOpType.add)

            bnc2 = dram.tile([1, 2], FP32)
            bnc2o = dram.tile([N_CORES, 2], FP32)
            nc.gpsimd.dma_start(bnc2[:], sb2[:])
            nc.gpsimd.collective_compute(
                "AllGather", mybir.AluOpType.bypass,
                replica_groups=groups,
                ins=[bnc2.opt()], outs=[bnc2o.opt()],
            )
            gs2 = st.tile([N_CORES, 2], FP32)
            nc.gpsimd.dma_start(gs2[:], bnc2o[:])
            pb2 = ps_s.tile([128, 2], FP32, tag="pb2")
            nc.tensor.matmul(pb2[:], ones8[:], gs2[:], start=True, stop=True)
            g2b = st.tile([128, 2], FP32)
            nc.vector.tensor_copy(g2b[:], pb2[:])

            # binary_scale s = (Sum|c - mean| - thr * n_out) / n_nonout;
            # the matmul uses u = wsim / s (binaries exactly +-1) and s is
            # applied at PSUM eviction, so also build invs = 1 / s.
            n_out = g2b[:, 0:1]
            s_ac = g2b[:, 1:2]
            t1 = st.tile([128, 1], FP32)
            z1 = st.tile([128, 1], FP32)
            inv = st.tile([128, 1], FP32)
            rz1 = st.tile([128, 1], FP32)
            s_b = st.tile([128, 1], FP32)
            invs = st.tile([128, 1], FP32)
            nc.vector.scalar_tensor_tensor(
                z1[:], n_out, thr[:, 0:1], s_ac,
                AluOpType.mult, AluOpType.subtract)
            nc.vector.tensor_scalar(t1[:], n_out, -1.0, float(N_ELEM),
                                    op0=AluOpType.mult, op1=AluOpType.add)
            # invs first: it gates the B phase (s_b is only needed at
            # eviction time, well after the first matmuls).
            nc.vector.reciprocal(rz1[:], z1[:])
            nc.vector.scalar_tensor_tensor(
                invs[:], t1[:], -1.0, rz1[:],
                AluOpType.mult, AluOpType.mult)
            nc.vector.reciprocal(inv[:], t1[:])
            nc.vector.scalar_tensor_tensor(
                s_b[:], z1[:], -1.0, inv[:],
                AluOpType.mult, AluOpType.mult)
            ps_s_cm.__exit__(None, None, None)

            # ---- phase B: u = sign(w) + outlier * (w/s - sign(w)), bf16 ---
            # Non-outlier stationary weights are EXACTLY +-1 (zero mantissa),
            # which cuts PE-array multiplier power enough to avoid the P0
            # sustained-power downclock; s is re-applied at PSUM eviction.
            wsim = []
            for k in range(KC):
                t = wsim_p.tile([128, D_OUT_SH], BF16, name=f"wsim{k}",
                                tag=f"wsim{k}")
                wsim.append(t)
            for k in range(KC):
                d = spB.tile([128, D_OUT_SH], FP32, tag="d")
                dm = spB.tile([128, D_OUT_SH], FP32, tag="dm")
                # chunk 0 in two half-width slices so the first matmuls
                # (m=0/1) can launch ~1us earlier
                slices = ([slice(0, 256), slice(256, 512)] if k == 0
                          else [slice(0, D_OUT_SH)])
                for sl in slices:
                    nc.vector.scalar_tensor_tensor(
                        d[:, sl], wt[k][:, sl], invs[:, 0:1], sg[k][:, sl],
                        AluOpType.mult, AluOpType.subtract)
                    nc.vector.scalar_tensor_tensor(
                        dm[:, sl], d[:, sl], 1.0, mout[k][:, sl],
                        AluOpType.mult, AluOpType.mult)
                    nc.vector.tensor_tensor(wsim[k][:, sl], sg[k][:, sl],
                                            dm[:, sl], op=AluOpType.add)

            # ---- phase C: dense bf16 matmul -------------------------------
            def xt_dma(tt, k):
                t = xp.tile([128, TOK_TILE], BF16, tag="xt")
                r0 = (tt * KC + k) * 128
                nc.sync.dma_start(t[:], xt_in[r0:r0 + 128, :])
                return t

            def evict(pt, tt, m):
                ot = op.tile([128, TOK_TILE], FP32, name=f"ot_{tt}_{m}",
                             tag="ot")
                # Last two token tiles: spread evictions across ScalarE and
                # VectorE and the out-DMAs across the gpsimd and sync queues
                # so the kernel tail isn't serialized on one engine/queue.
                late = tt >= N_TOKT - 2
                if late and m % 2 == 1:
                    nc.vector.tensor_scalar(ot[:], pt[:], s_b[:, 0:1],
                                            bias_sb[:, m:m + 1],
                                            op0=AluOpType.mult,
                                            op1=AluOpType.add)
                else:
                    nc.scalar.activation(ot[:], pt[:], F.Identity,
                                         scale=s_b[:, 0:1],
                                         bias=bias_sb[:, m:m + 1])
                # Keep the gpsimd queue empty at the end so its ~7us DRAIN
                # (TileContext exit) hides under the main phase.
                if late:
                    dma_eng = nc.sync if m < 2 else nc.scalar
                else:
                    dma_eng = nc.gpsimd
                dma_eng.dma_start(
                    out_t[m * 128:(m + 1) * 128,
                          tt * TOK_TILE:(tt + 1) * TOK_TILE], ot[:])

            with (
                tc.tile_pool(name="ops", bufs=2, space="PSUM") as pp,
            ):
                # The first two token tiles run k-interleaved on both PSUM
                # buffer sets (8 banks): consumption is then 8 matmuls per
                # wsim chunk (~2.1 us), matching VectorE's build rate so the
                # binarize never stalls TensorE.
                ps0 = [pp.tile([128, TOK_TILE], FP32, name=f"ps_0_{m}",
                               tag=f"ps{m}") for m in range(MSUB)]
                ps1 = [pp.tile([128, TOK_TILE], FP32, name=f"ps_1_{m}",
                               tag=f"ps{m}") for m in range(MSUB)]
                for k in range(KC):
                    xa = xt_dma(0, k)
                    xb = xt_dma(1, k)
                    for m in range(MSUB):
                        nc.tensor.matmul(ps0[m][:],
                                         wsim[k][:, m * 128:(m + 1) * 128],
                                         xa[:],
                                         start=(k == 0), stop=(k == KC - 1))
                    for m in range(MSUB):
                        nc.tensor.matmul(ps1[m][:],
                                         wsim[k][:, m * 128:(m + 1) * 128],
                                         xb[:],
                                         start=(k == 0), stop=(k == KC - 1))
                for m in range(MSUB):
                    evict(ps0[m], 0, m)
                for m in range(MSUB):
                    evict(ps1[m], 1, m)

                for tt in range(2, N_TOKT):
                    psum = [pp.tile([128, TOK_TILE], FP32, name=f"ps_{tt}_{m}",
                                    tag=f"ps{m}")
                            for m in range(MSUB)]
                    for k in range(KC):
                        xt_t = xt_dma(tt, k)
                        for m in range(MSUB):
                            nc.tensor.matmul(
                                psum[m][:],
                                wsim[k][:, m * 128:(m + 1) * 128],
                                xt_t[:],
                                start=(k == 0), stop=(k == KC - 1))
                    for m in range(MSUB):
                        evict(psum[m], tt, m)

    return nc


_NC_CACHE = None


def _get_program():
    global _NC_CACHE
    if _NC_CACHE is None:
        _NC_CACHE = _build_program()
    return _NC_CACHE


def _make_in_maps(x, weight, bias):
    # [tt, k, p, j] block layout: tile (tt, k) is contiguous in DRAM.
    xT = np.ascontiguousarray(
        x.reshape(N_TOKT, TOK_TILE, KC, 128).astype(ml_dtypes.bfloat16)
        .transpose(0, 2, 3, 1)).reshape(N_TOKT * D_IN, TOK_TILE)
    in_maps = []
    for c in range(N_CORES):
        o0 = c * D_OUT_SH
        wT_c = np.ascontiguousarray(weight[o0:o0 + D_OUT_SH, :].T)  # [D_IN, 512]
        b_c = np.ascontiguousarray(
            bias[o0:o0 + D_OUT_SH].reshape(MSUB, 128).T)  # [128, MSUB]
        in_maps.append({"xt": xT, "wt": wT_c, "bias": b_c})
    return in_maps


def kernel(x: np.ndarray, weight: np.ndarray, bias: np.ndarray) -> np.ndarray:
    nc = _get_program()
    in_maps = _make_in_maps(x, weight, bias)
    res = run_bass_kernel_spmd(nc, in_maps, list(range(N_CORES)))
    outT = np.concatenate([res.results[c]["out"] for c in range(N_CORES)], axis=0)
    return np.ascontiguousarray(outT.T).reshape(x.shape[0], x.shape[1], D_OUT)
